# revision 1
# baseline (speedup 1.0000x reference)
"""MultiHeadAttention (qk-LayerNorm + RoPE) Trainium2 kernel, 8 NeuronCores.

Sharding: batch (4) x head-group (2x8 heads). Core c handles batch c//2,
heads 8*(c%2) .. 8*(c%2)+7. Each core computes QKV projections for its
batch restricted to its head group, per-head LayerNorm + rotary embedding,
attention, and a partial output projection over its 512 context channels.
A pairwise ReduceScatter sums the two partial o_proj results per batch and
leaves each core with half the rows; the host concatenates.

All matmuls run as float32r (tf32-like) on the PE. Attention is computed
with scores transposed ([s, t] layout) so softmax normalization can be
deferred: ctx_unnorm and sum-of-exp come from one matmul with a ones row
appended to V, and the per-token reciprocal is broadcast across partitions
with a K=1 matmul.
"""
import sys

for _p in ("/opt/trn_rl_repo", "/root/.axon_site", "/root/.axon_site/_ro/trn_rl_repo",
           "/root/.axon_site/_ro/pypackages"):
    if _p not in sys.path:
        sys.path.append(_p)

import numpy as np

import concourse.bass as bass
import concourse.tile as tile
from concourse import bacc, mybir
from concourse.bass_utils import run_bass_kernel_spmd
from concourse.masks import make_identity

F32 = mybir.dt.float32
F32R = mybir.dt.float32r
P = 128
B, L, C, H, D = 4, 1024, 1024, 16, 64
HC = 8          # heads per core
CG = HC * D     # 512 context channels per core
NT = L // P     # 8 token tiles
NCK = C // P    # 8 contraction tiles
THETA = 50000.0
EPS = 1e-5

_NC_CACHE = {}


def _build_nc():
    nc = bacc.Bacc("TRN2", target_bir_lowering=False, debug=False, num_devices=8)

    xT_d = nc.dram_tensor("xT", [C, L], F32, kind="ExternalInput")
    wqT_d = nc.dram_tensor("wqT", [C, CG], F32, kind="ExternalInput")
    wkT_d = nc.dram_tensor("wkT", [C, CG], F32, kind="ExternalInput")
    wvT_d = nc.dram_tensor("wvT", [C, CG], F32, kind="ExternalInput")
    woT_d = nc.dram_tensor("woT", [D, HC, C], F32, kind="ExternalInput")
    aq_d = nc.dram_tensor("aq", [L, D], F32, kind="ExternalInput")
    bq_d = nc.dram_tensor("bq", [L, D], F32, kind="ExternalInput")
    ak_d = nc.dram_tensor("ak", [L, D], F32, kind="ExternalInput")
    bk_d = nc.dram_tensor("bk", [L, D], F32, kind="ExternalInput")
    ones_d = nc.dram_tensor("ones64", [1, D], F32, kind="ExternalInput")
    out_d = nc.dram_tensor("out", [L // 2, C], F32, kind="ExternalOutput")

    with tile.TileContext(nc) as tc:
        with (
            tc.tile_pool(name="const", bufs=1) as constp,
            tc.tile_pool(name="w", bufs=1) as wpool,
            tc.tile_pool(name="big", bufs=1) as bigp,
            tc.tile_pool(name="xt", bufs=2) as xtp,
            tc.tile_pool(name="sq", bufs=1) as sqp,
            tc.tile_pool(name="scr", bufs=2) as scrp,
            tc.tile_pool(name="rope", bufs=2) as ropep,
            tc.tile_pool(name="stat", bufs=2) as statp,
            tc.tile_pool(name="exp", bufs=2) as expp,
            tc.tile_pool(name="fin", bufs=2) as finp,
            tc.tile_pool(name="dram", bufs=1, space="DRAM") as dram,
        ):
            ident = constp.tile([P, P], F32)
            make_identity(nc, ident)
            eps_t = constp.tile([P, 1], F32)
            nc.vector.memset(eps_t[:], EPS)
            ones_row = constp.tile([65, D], F32R)
            nc.sync.dma_start(ones_row[64:65, :], ones_d.ap().bitcast(F32R))

            aq_t = constp.tile([P, NT, D], F32)
            nc.sync.dma_start(aq_t[:], aq_d.ap().rearrange("(t p) d -> p t d", p=P))
            bq_t = constp.tile([P, NT, D], F32)
            nc.sync.dma_start(bq_t[:], bq_d.ap().rearrange("(t p) d -> p t d", p=P))
            ak_t = constp.tile([P, NT, D], F32)
            nc.sync.dma_start(ak_t[:], ak_d.ap().rearrange("(t p) d -> p t d", p=P))
            bk_t = constp.tile([P, NT, D], F32)
            nc.sync.dma_start(bk_t[:], bk_d.ap().rearrange("(t p) d -> p t d", p=P))

            # per-ck weight tiles so the first matmuls start after ~1.5MB of DMA
            wq_t, wk_t, wv_t = [], [], []
            for ck in range(NCK):
                for lst, nm, d_ in ((wq_t, "wq", wqT_d), (wk_t, "wk", wkT_d),
                                    (wv_t, "wv", wvT_d)):
                    t_ = wpool.tile([P, CG], F32R, tag=f"{nm}{ck}", name=f"{nm}{ck}")
                    nc.sync.dma_start(
                        t_[:],
                        d_.ap().rearrange("(k p) o -> p k o", p=P)[:, ck, :].bitcast(F32R))
                    lst.append(t_)

            # v with a ones column appended per head: [s_tile, j, head, 65]
            v_sb = bigp.tile([P, NT, HC, D + 1], F32R)
            nc.sync.dma_start(
                v_sb[:, :, :, D:D + 1].rearrange("p t h o -> p (t h) o"),
                ones_d.ap()[0:1, 0:1].rearrange("a b -> a b ()").to_broadcast(
                    (P, NT * HC, 1)).bitcast(F32R),
            )
            qT_pack = bigp.tile([P, HC // 2, L], F32R)
            kT_pack = bigp.tile([P, HC // 2, L], F32R)
            ctxT = bigp.tile([D, HC, L], F32R)

            # ---------------- Phase 1: QKV + LN + RoPE + transpose ----------
            with tc.tile_pool(name="ps1", bufs=2, space="PSUM") as ps1, \
                 tc.tile_pool(name="pst", bufs=2, space="PSUM") as pst:
                for ti in range(NT):
                    xt = xtp.tile([P, NCK, P], F32R)
                    nc.sync.dma_start(
                        xt[:],
                        xT_d.ap().rearrange("(k p) t -> p k t", p=P)[:, :, bass.ts(ti, P)].bitcast(F32R),
                    )
                    psq = ps1.tile([P, CG], F32, tag="psq")
                    psk = ps1.tile([P, CG], F32, tag="psk")
                    psv = ps1.tile([P, CG], F32, tag="psv")
                    for ps_, w_ in ((psq, wq_t), (psk, wk_t), (psv, wv_t)):
                        for ck in range(NCK):
                            nc.tensor.matmul(ps_[:], xt[:, ck, :], w_[ck][:],
                                             start=(ck == 0), stop=(ck == NCK - 1))

                    # v straight to SBUF (rounded to f32r); ACT engine to keep DVE free
                    nc.scalar.copy(
                        v_sb[:, ti, :, 0:D],
                        psv[:].rearrange("p (h d) -> p h d", d=D))

                    # LN stats for q and k: sums and sums of squares
                    stats = statp.tile([P, 4, HC], F32)
                    for i, ps_ in enumerate((psq, psk)):
                        nc.vector.reduce_sum(
                            stats[:, 2 * i, :], ps_[:].rearrange("p (h d) -> p h d", d=D),
                            axis=mybir.AxisListType.X)
                        sq = sqp.tile([P, CG], F32)
                        nc.scalar.square(sq[:], ps_[:])
                        nc.vector.reduce_sum(
                            stats[:, 2 * i + 1, :], sq[:].rearrange("p (h d) -> p h d", d=D),
                            axis=mybir.AxisListType.X)
                    mus = statp.tile([P, 2, HC], F32)
                    nc.vector.tensor_scalar_mul(mus[:], stats[:, 0::2, :], 1.0 / D)
                    ms2 = statp.tile([P, 2, HC], F32)
                    nc.vector.tensor_scalar_mul(ms2[:], stats[:, 1::2, :], 1.0 / D)
                    var = statp.tile([P, 2, HC], F32)
                    nc.vector.tensor_mul(var[:], mus[:], mus[:])
                    nc.vector.tensor_sub(var[:], ms2[:], var[:])
                    std = statp.tile([P, 2, HC], F32)
                    nc.scalar.activation(std[:], var[:], mybir.ActivationFunctionType.Sqrt,
                                         bias=eps_t[:])
                    invstd = statp.tile([P, 2, HC], F32)
                    nc.vector.reciprocal(invstd[:], std[:])
                    shift = statp.tile([P, 2, HC], F32)
                    nc.vector.tensor_mul(shift[:], mus[:], invstd[:])

                    for i, (ps_, a_t, b_t, dstpack) in enumerate(
                            ((psq, aq_t, bq_t, qT_pack), (psk, ak_t, bk_t, kT_pack))):
                        inv_b = invstd[:, i, :].rearrange("p h -> p h ()").to_broadcast((P, HC, D))
                        sh_b = shift[:, i, :].rearrange("p h -> p h ()").to_broadcast((P, HC, D))
                        a_b = a_t[:, ti, :].rearrange("p d -> p () d").to_broadcast((P, HC, D))
                        t1 = scrp.tile([P, HC, D], F32, tag="t1")
                        nc.vector.tensor_mul(t1[:], ps_[:].rearrange("p (h d) -> p h d", d=D), inv_b)
                        nc.vector.tensor_sub(t1[:], t1[:], sh_b)
                        rope = ropep.tile([P, HC, D], F32, tag=f"rope{i}")
                        nc.vector.tensor_mul(rope[:], t1[:], a_b)
                        r2 = scrp.tile([P, HC, D], F32, tag="r2")
                        h_ = D // 2
                        nc.vector.tensor_mul(
                            r2[:, :, 0:h_], t1[:, :, h_:D],
                            b_t[:, ti, 0:h_].rearrange("p d -> p () d").to_broadcast((P, HC, h_)))
                        nc.vector.tensor_mul(
                            r2[:, :, h_:D], t1[:, :, 0:h_],
                            b_t[:, ti, h_:D].rearrange("p d -> p () d").to_broadcast((P, HC, h_)))
                        nc.vector.tensor_add(rope[:], rope[:], r2[:])
                        for pr in range(HC // 2):
                            ps_t = pst.tile([P, P], F32)
                            nc.tensor.transpose(
                                ps_t[:],
                                rope[:, 2 * pr:2 * pr + 2, :].rearrange("p h d -> p (h d)"),
                                ident[:])
                            nc.scalar.copy(dstpack[:, pr, bass.ts(ti, P)], ps_t[:])

            # ---------------- Phase 2: attention per head -------------------
            with tc.tile_pool(name="pss", bufs=2, space="PSUM") as pssp, \
                 tc.tile_pool(name="psc", bufs=1, space="PSUM") as pscp, \
                 tc.tile_pool(name="psr", bufs=1, space="PSUM") as psrp:
                for h in range(HC):
                    pr, sub = h // 2, h % 2
                    lo, hi = D * sub, D * sub + D
                    psc = pscp.tile([D + 1, L], F32)
                    for j in range(NT):
                        pss = pssp.tile([P, L], F32)
                        for m in range(2):
                            nc.tensor.matmul(
                                pss[:, bass.ts(m, 512)],
                                kT_pack[lo:hi, pr, bass.ts(j, P)],
                                qT_pack[lo:hi, pr, bass.ts(m, 512)],
                                start=True, stop=True)
                        expT = expp.tile([P, L], F32R)
                        nc.scalar.activation(expT[:], pss[:],
                                             mybir.ActivationFunctionType.Exp,
                                             scale=float(D) ** -0.5)
                        for m in range(2):
                            nc.tensor.matmul(
                                psc[:, bass.ts(m, 512)],
                                v_sb[:, j, h, :],
                                expT[:, bass.ts(m, 512)],
                                start=(j == 0), stop=(j == NT - 1))
                    recip = finp.tile([D + 1, L], F32R, tag="recip")
                    with nc.allow_low_precision(reason="f32r rounding for rb matmul"):
                        nc.vector.reciprocal(recip[D:D + 1, :], psc[D:D + 1, :])
                    ps_rb = psrp.tile([D, L], F32)
                    for m in range(2):
                        nc.tensor.matmul(
                            ps_rb[:, bass.ts(m, 512)],
                            ones_row[64:65, :],
                            recip[D:D + 1, bass.ts(m, 512)],
                            start=True, stop=True)
                    for m in range(2):
                        rb_sb = finp.tile([D, 512], F32, tag="rb")
                        nc.vector.tensor_copy(rb_sb[:], ps_rb[:, bass.ts(m, 512)])
                        nc.vector.tensor_mul(ctxT[:, h, bass.ts(m, 512)],
                                             psc[0:D, bass.ts(m, 512)], rb_sb[:])

            # ---------------- Phase 3: output projection --------------------
            # wo reuses the per-ck wq slots (dead after phase 1)
            wo_l = []
            for h in range(HC):
                wo_h = wpool.tile([D, C], F32R, tag=f"wq{h}", name=f"wo{h}")
                nc.sync.dma_start(wo_h[:], woT_d.ap()[:, h, :].bitcast(F32R))
                wo_l.append(wo_h)

            bounce_in = [dram.tile([L // 2, C], F32, tag=f"bin{i}", name=f"bin{i}")
                         for i in range(2)]
            bounce_out = [dram.tile([L // 4, C], F32, tag=f"bout{i}", name=f"bout{i}")
                         for i in range(2)]

            def emit_rs(half):
                nc.gpsimd.collective_compute(
                    "ReduceScatter",
                    mybir.AluOpType.add,
                    replica_groups=[[0, 1], [2, 3], [4, 5], [6, 7]],
                    ins=[bounce_in[half][:].opt()],
                    outs=[bounce_out[half][:].opt()],
                )
                nc.sync.dma_start(out_d.ap()[bass.ts(half, L // 4), :],
                                  bounce_out[half][:])

            with tc.tile_pool(name="pso", bufs=2, space="PSUM") as psop:
                for ti in range(NT):
                    pso = psop.tile([P, C], F32)
                    for m in range(2):
                        for h in range(HC):
                            nc.tensor.matmul(
                                pso[:, bass.ts(m, 512)],
                                ctxT[:, h, bass.ts(ti, P)],
                                wo_l[h][:, bass.ts(m, 512)],
                                start=(h == 0), stop=(h == HC - 1))
                    out_sb = finp.tile([P, C], F32, tag="out", bufs=1)
                    nc.vector.tensor_copy(out_sb[:], pso[:])
                    nc.sync.dma_start(bounce_in[ti // 4][bass.ts(ti % 4, P), :], out_sb[:])
                    if ti == NT // 2 - 1:
                        emit_rs(0)
                emit_rs(1)

    nc.compile()
    return nc


def _rope_tables(w, b):
    """A[t,d], B[t,d] with the rotate-half sign folded into B."""
    inv_freq = 1.0 / THETA ** (np.arange(0, D, 2, dtype=np.float64) / D)
    freqs = np.arange(L, dtype=np.float64)[:, None] * inv_freq[None, :]
    freqs = np.concatenate([freqs, freqs], axis=1)           # [L, D]
    cos, sin = np.cos(freqs), np.sin(freqs)
    w = w.astype(np.float64)
    w_rot = np.concatenate([w[D // 2:], w[:D // 2]])
    sgn = np.concatenate([-np.ones(D // 2), np.ones(D // 2)])
    A = (cos * w[None, :]).astype(np.float32)
    Bt = (sin * w_rot[None, :] * sgn[None, :]).astype(np.float32)
    if np.any(b != 0):
        raise NotImplementedError("nonzero qk-norm bias not supported")
    return A, Bt


def kernel(**inputs):
    x = np.asarray(inputs["q"], dtype=np.float32)
    Wq = np.asarray(inputs["Wq"], dtype=np.float32)
    Wk = np.asarray(inputs["Wk"], dtype=np.float32)
    Wv = np.asarray(inputs["Wv"], dtype=np.float32)
    Wo = np.asarray(inputs["Wo"], dtype=np.float32)
    bo = np.asarray(inputs["bo"], dtype=np.float32)
    assert not np.any(bo != 0), "nonzero output bias not supported"

    Aq, Bq = _rope_tables(np.asarray(inputs["qn_w"], np.float32),
                          np.asarray(inputs["qn_b"], np.float32))
    Ak, Bk = _rope_tables(np.asarray(inputs["kn_w"], np.float32),
                          np.asarray(inputs["kn_b"], np.float32))
    ones64 = np.ones((1, D), dtype=np.float32)
    WoT = np.ascontiguousarray(Wo.T)                          # [C(c'), C(o)]

    if "nc" not in _NC_CACHE:
        _NC_CACHE["nc"] = _build_nc()
    nc = _NC_CACHE["nc"]

    in_maps = []
    for c in range(8):
        b_, g = c // 2, c % 2
        sl = slice(g * CG, (g + 1) * CG)
        in_maps.append({
            "xT": np.ascontiguousarray(x[b_].T),
            "wqT": np.ascontiguousarray(Wq[sl, :].T),
            "wkT": np.ascontiguousarray(Wk[sl, :].T),
            "wvT": np.ascontiguousarray(Wv[sl, :].T),
            "woT": np.ascontiguousarray(
                WoT[sl, :].reshape(HC, D, C).transpose(1, 0, 2)),
            "aq": Aq, "bq": Bq, "ak": Ak, "bk": Bk,
            "ones64": ones64,
        })

    res = run_bass_kernel_spmd(nc, in_maps, core_ids=list(range(8)))
    # two half-ReduceScatters: each core's "out" holds [rank's quarter of rows
    # 0:512 ; rank's quarter of rows 512:1024]
    Q = L // 4
    out = np.empty((B, L, C), dtype=np.float32)
    for b_ in range(B):
        ev, od = res.results[2 * b_]["out"], res.results[2 * b_ + 1]["out"]
        out[b_, 0 * Q:1 * Q] = ev[0:Q]
        out[b_, 1 * Q:2 * Q] = od[0:Q]
        out[b_, 2 * Q:3 * Q] = ev[Q:2 * Q]
        out[b_, 3 * Q:4 * Q] = od[Q:2 * Q]
    return out



# revision 13
# speedup vs baseline: 1.4825x; 1.4825x over previous
"""MultiHeadAttention (qk-LayerNorm + RoPE) Trainium2 kernel, 8 NeuronCores.

Sharding: batch (4) x query-half (2x512 tokens), collective-free. Core c
handles batch c//2, query rows (c%2)*512 .. +512, ALL 16 heads. K/V
projections for the full 1024-token sequence are duplicated within each
batch pair; in exchange there are no collectives at all (the multi-device
rendezvous barrier plus ReduceScatter cost ~120us in the pair-parallel
variant). Each core writes its 512 output rows directly; the host
concatenates.

All matmuls run in bf16 (1 col/cycle on the PE, half the DMA/SBUF of
f32r). LayerNorm means are folded into Wq/Wk on the host (per-head
row-block mean subtraction makes projections zero-mean), so only the
variance is computed on device. D^-0.5 is folded into the q rope tables.
Scores are computed transposed ([s, t]) so the softmax denominator comes
free from a ones column appended to V; normalization uses one
reciprocal_approx_fast over head-stacked sums and a DMA partition
broadcast. o_proj packs head pairs (2x64 rows) for full 128-deep
contraction.
"""
import sys

for _p in ("/opt/trn_rl_repo", "/root/.axon_site", "/root/.axon_site/_ro/trn_rl_repo",
           "/root/.axon_site/_ro/pypackages"):
    if _p not in sys.path:
        sys.path.append(_p)

import numpy as np
import ml_dtypes

import concourse.bass as bass
import concourse.tile as tile
from concourse import bacc, mybir
from concourse.bass_utils import run_bass_kernel_spmd
from concourse.masks import make_identity

BF16_NP = ml_dtypes.bfloat16
F32 = mybir.dt.float32
BF16 = mybir.dt.bfloat16
P = 128
B, L, C, H, D = 4, 1024, 1024, 16, 64
LQ = L // 2          # query rows per core
NTQ = LQ // P        # 4 query token tiles
NT = L // P          # 8 key token tiles
NCK = C // P         # 8 contraction tiles
NPR = H // 2         # 8 head pairs
THETA = 50000.0
EPS = 1e-5

_NC_CACHE = {}


def _build_nc():
    nc = bacc.Bacc("TRN2", target_bir_lowering=False, debug=False, num_devices=8)

    xT_d = nc.dram_tensor("xT", [C, L], BF16, kind="ExternalInput")
    xqT_d = nc.dram_tensor("xqT", [C, LQ], BF16, kind="ExternalInput")
    wqT_d = nc.dram_tensor("wqT", [C, C], BF16, kind="ExternalInput")
    wkT_d = nc.dram_tensor("wkT", [C, C], BF16, kind="ExternalInput")
    wvT_d = nc.dram_tensor("wvT", [C, C], BF16, kind="ExternalInput")
    woP_d = nc.dram_tensor("woP", [NPR, P, C], BF16, kind="ExternalInput")
    aq_d = nc.dram_tensor("aq", [LQ, D], BF16, kind="ExternalInput")
    bq_d = nc.dram_tensor("bq", [LQ, D], BF16, kind="ExternalInput")
    ak_d = nc.dram_tensor("ak", [L, D], BF16, kind="ExternalInput")
    bk_d = nc.dram_tensor("bk", [L, D], BF16, kind="ExternalInput")
    out_d = nc.dram_tensor("out", [LQ, C], F32, kind="ExternalOutput")

    with tile.TileContext(nc) as tc:
        with (
            tc.tile_pool(name="const", bufs=1) as constp,
            tc.tile_pool(name="w", bufs=1) as wpool,
            tc.tile_pool(name="big", bufs=1) as bigp,
            tc.tile_pool(name="xt", bufs=2) as xtp,
            tc.tile_pool(name="stg", bufs=2) as stgp,
            tc.tile_pool(name="stat", bufs=2) as statp,
            tc.tile_pool(name="exp", bufs=2) as expp,
            tc.tile_pool(name="fin", bufs=2) as finp,
            tc.tile_pool(name="dram", bufs=1, space="DRAM") as dramp,
        ):
            ident = constp.tile([P, P], BF16)
            make_identity(nc, ident)
            eps_t = constp.tile([P, 1], F32)
            nc.vector.memset(eps_t[:], EPS)
            ones_row = constp.tile([1, D], BF16)
            nc.vector.memset(ones_row[:], 1.0)

            aq_t = constp.tile([P, NTQ, D], BF16)
            nc.sync.dma_start(aq_t[:], aq_d.ap().rearrange("(t p) d -> p t d", p=P))
            bq_t = constp.tile([P, NTQ, D], BF16)
            nc.sync.dma_start(bq_t[:], bq_d.ap().rearrange("(t p) d -> p t d", p=P))
            ak_t = constp.tile([P, NT, D], BF16)
            nc.sync.dma_start(ak_t[:], ak_d.ap().rearrange("(t p) d -> p t d", p=P))
            bk_t = constp.tile([P, NT, D], BF16)
            nc.sync.dma_start(bk_t[:], bk_d.ap().rearrange("(t p) d -> p t d", p=P))

            # per-ck weight tiles; q weights first so phase Q starts early
            wq_t, wk_t, wv_t = [], [], []
            for lst, nm, d_ in ((wq_t, "wq", wqT_d), (wk_t, "wk", wkT_d),
                                (wv_t, "wv", wvT_d)):
                for ck in range(NCK):
                    t_ = wpool.tile([P, C], BF16, tag=f"{nm}{ck}", name=f"{nm}{ck}")
                    nc.sync.dma_start(
                        t_[:], d_.ap().rearrange("(k p) o -> p k o", p=P)[:, ck, :])
                    lst.append(t_)
            wo_l = []
            for pr in range(NPR):
                t_ = wpool.tile([P, C], BF16, tag=f"wo{pr}", name=f"wo{pr}")
                nc.sync.dma_start(t_[:], woP_d.ap()[pr])
                wo_l.append(t_)

            # v with a ones column appended per head: [s, j, h, 65]
            v_sb = bigp.tile([P, NT, H, D + 1], BF16)
            nc.gpsimd.memset(v_sb[:, :, :, D:D + 1], 1.0)

            qT_pack = bigp.tile([P, NPR, LQ], BF16)
            kT_pack = bigp.tile([P, NPR, L], BF16)
            ctxT = bigp.tile([P, NPR, LQ], BF16)

            def ln_rope(ps, a_t, b_t, ti, dst_pack, pst_pool):
                """psum [t,1024] f32 -> LN(var-only)+rope -> transpose into
                dst_pack[:, pr, ti*128:...]. Returns nothing."""
                qs = stgp.tile([P, H, D], BF16, tag="stg")
                nc.scalar.copy(qs[:], ps[:].rearrange("p (h d) -> p h d", d=D))
                sq = stgp.tile([P, H, D], BF16, tag="sq")
                nc.vector.tensor_mul(sq[:], qs[:], qs[:])
                ss = statp.tile([P, H], F32, tag="ss")
                nc.vector.reduce_sum(ss[:], sq[:], axis=mybir.AxisListType.X)
                std = statp.tile([P, H], F32, tag="std")
                nc.scalar.activation(std[:], ss[:], mybir.ActivationFunctionType.Sqrt,
                                     bias=eps_t[:], scale=1.0 / D)
                inv = statp.tile([P, H], F32, tag="inv")
                nc.vector.reciprocal(inv[:], std[:])
                invb = statp.tile([P, H], BF16, tag="invb")
                nc.vector.tensor_copy(invb[:], inv[:])

                a_b = a_t[:, ti, :].rearrange("p d -> p () d").to_broadcast((P, H, D))
                r = stgp.tile([P, H, D], BF16, tag="r")
                nc.vector.tensor_mul(r[:], qs[:], a_b)
                r2 = stgp.tile([P, H, D], BF16, tag="r2")
                h_ = D // 2
                nc.vector.tensor_mul(
                    r2[:, :, 0:h_], qs[:, :, h_:D],
                    b_t[:, ti, 0:h_].rearrange("p d -> p () d").to_broadcast((P, H, h_)))
                nc.vector.tensor_mul(
                    r2[:, :, h_:D], qs[:, :, 0:h_],
                    b_t[:, ti, h_:D].rearrange("p d -> p () d").to_broadcast((P, H, h_)))
                nc.vector.tensor_add(r[:], r[:], r2[:])
                nc.vector.tensor_mul(
                    r[:], r[:],
                    invb[:].rearrange("p h -> p h ()").to_broadcast((P, H, D)))

                for grp in range(2):
                    pst = pst_pool.tile([P, 4, P], BF16, tag="pst")
                    for q4 in range(4):
                        pr = grp * 4 + q4
                        nc.tensor.transpose(
                            pst[:, q4, :],
                            r[:, 2 * pr:2 * pr + 2, :].rearrange("p h d -> p (h d)"),
                            ident[:])
                    nc.vector.tensor_copy(
                        dst_pack[:, 4 * grp:4 * grp + 4, bass.ts(ti, P)], pst[:])

            # ---------------- Phase Q: q projection + LN + RoPE -------------
            with tc.tile_pool(name="psq", bufs=2, space="PSUM") as psqp, \
                 tc.tile_pool(name="pstq", bufs=2, space="PSUM") as pstqp:
                for ti in range(NTQ):
                    xt = xtp.tile([P, NCK, P], BF16, tag="xq")
                    nc.sync.dma_start(
                        xt[:],
                        xqT_d.ap().rearrange("(k p) t -> p k t", p=P)[:, :, bass.ts(ti, P)])
                    psq = psqp.tile([P, C], F32)
                    for ch in range(2):
                        for ck in range(NCK):
                            nc.tensor.matmul(psq[:, bass.ts(ch, 512)],
                                             xt[:, ck, :],
                                             wq_t[ck][:, bass.ts(ch, 512)],
                                             start=(ck == 0), stop=(ck == NCK - 1))
                    ln_rope(psq, aq_t, bq_t, ti, qT_pack, pstqp)

            # ---------------- Phase KV: k/v projection + LN + RoPE ----------
            with tc.tile_pool(name="psk", bufs=2, space="PSUM") as pskp, \
                 tc.tile_pool(name="psv", bufs=1, space="PSUM") as psvp, \
                 tc.tile_pool(name="pstk", bufs=2, space="PSUM") as pstkp:
                for ti in range(NT):
                    xt = xtp.tile([P, NCK, P], BF16, tag="xk")
                    nc.sync.dma_start(
                        xt[:],
                        xT_d.ap().rearrange("(k p) t -> p k t", p=P)[:, :, bass.ts(ti, P)])
                    psk = pskp.tile([P, C], F32)
                    psv = psvp.tile([P, C], F32)
                    for ps_, w_ in ((psk, wk_t), (psv, wv_t)):
                        for ch in range(2):
                            for ck in range(NCK):
                                nc.tensor.matmul(ps_[:, bass.ts(ch, 512)],
                                                 xt[:, ck, :],
                                                 w_[ck][:, bass.ts(ch, 512)],
                                                 start=(ck == 0), stop=(ck == NCK - 1))
                    nc.scalar.copy(
                        v_sb[:, ti, :, 0:D],
                        psv[:].rearrange("p (h d) -> p h d", d=D))
                    ln_rope(psk, ak_t, bk_t, ti, kT_pack, pstkp)

            # ---------------- Phase ATT: attention, 4 heads per round -------
            with tc.tile_pool(name="pss", bufs=1, space="PSUM") as pssp, \
                 tc.tile_pool(name="psc", bufs=1, space="PSUM") as pscp:
                for g in range(4):
                    heads = [4 * g + i for i in range(4)]
                    psc = [pscp.tile([D + 1, LQ], F32, tag=f"c{i}", name=f"psc{g}_{i}")
                           for i in range(4)]
                    for j in range(NT):
                        pss = pssp.tile([P, 4, LQ], F32, tag="pss")
                        for i, h in enumerate(heads):
                            lo = (h % 2) * D
                            nc.tensor.matmul(
                                pss[:, i, :],
                                kT_pack[lo:lo + D, h // 2, bass.ts(j, P)],
                                qT_pack[lo:lo + D, h // 2, :],
                                start=True, stop=True)
                        expT = expp.tile([P, 4, LQ], BF16, tag="expT")
                        nc.scalar.activation(expT[:], pss[:],
                                             mybir.ActivationFunctionType.Exp)
                        for i, h in enumerate(heads):
                            nc.tensor.matmul(
                                psc[i][:],
                                v_sb[:, j, h, :],
                                expT[:, i, :],
                                start=(j == 0), stop=(j == NT - 1))
                    # drain: unnormalized ctx (packed 2 heads / 128p) + recips
                    smq = finp.tile([1, 4, LQ], F32, tag="smq")
                    for i, h in enumerate(heads):
                        lo = (h % 2) * D
                        nc.vector.tensor_copy(
                            ctxT[lo:lo + D, h // 2, :], psc[i][0:D, :])
                        nc.vector.tensor_copy(smq[0:1, i, :], psc[i][D:D + 1, :])
                    rbqb = finp.tile([1, 4, LQ], BF16, tag="rbqb")
                    with nc.allow_low_precision(reason="softmax recip to bf16"):
                        nc.vector.reciprocal(rbqb[:], smq[:])
                    # broadcast 1/sums across 64 partitions via K=1 matmul
                    # (reuses freed psc bank slots), then normalize in place
                    for i, h in enumerate(heads):
                        lo = (h % 2) * D
                        rb_ps = pscp.tile([D, LQ], F32, tag=f"c{i}",
                                          name=f"rbps{g}_{i}")
                        nc.tensor.matmul(rb_ps[:], ones_row[:],
                                         rbqb[0:1, i, :], start=True, stop=True)
                        nc.vector.tensor_mul(ctxT[lo:lo + D, h // 2, :],
                                             ctxT[lo:lo + D, h // 2, :],
                                             rb_ps[:])

            # ---------------- Phase O: output projection --------------------
            with tc.tile_pool(name="pso", bufs=2, space="PSUM") as psop:
                for ti in range(NTQ):
                    pso = psop.tile([P, C], F32)
                    for pr in range(NPR):
                        for ch in range(2):
                            nc.tensor.matmul(
                                pso[:, bass.ts(ch, 512)],
                                ctxT[:, pr, bass.ts(ti, P)],
                                wo_l[pr][:, bass.ts(ch, 512)],
                                start=(pr == 0), stop=(pr == NPR - 1))
                    out_sb = finp.tile([P, C], F32, tag="out")
                    nc.vector.tensor_copy(out_sb[:], pso[:])
                    nc.sync.dma_start(out_d.ap()[bass.ts(ti, P), :], out_sb[:])

    nc.compile()
    return nc


def _rope_tables(w, b, length, scale):
    """A[t,d], B[t,d] with rotate-half sign and LN weight folded in."""
    inv_freq = 1.0 / THETA ** (np.arange(0, D, 2, dtype=np.float64) / D)
    freqs = np.arange(length, dtype=np.float64)[:, None] * inv_freq[None, :]
    freqs = np.concatenate([freqs, freqs], axis=1)
    cos, sin = np.cos(freqs), np.sin(freqs)
    w = w.astype(np.float64)
    w_rot = np.concatenate([w[D // 2:], w[:D // 2]])
    sgn = np.concatenate([-np.ones(D // 2), np.ones(D // 2)])
    A = (cos * w[None, :] * scale).astype(BF16_NP)
    Bt = (sin * w_rot[None, :] * sgn[None, :] * scale).astype(BF16_NP)
    if np.any(b != 0):
        raise NotImplementedError("nonzero qk-norm bias not supported")
    return A, Bt


def _fold_mean(W):
    """Remove per-head row-block mean: projections become zero-mean."""
    W = W.astype(np.float64).copy()
    for h in range(H):
        W[h * D:(h + 1) * D, :] -= W[h * D:(h + 1) * D, :].mean(0, keepdims=True)
    return W


def kernel(**inputs):
    x = np.asarray(inputs["q"], dtype=np.float32)
    Wq = np.asarray(inputs["Wq"], dtype=np.float32)
    Wk = np.asarray(inputs["Wk"], dtype=np.float32)
    Wv = np.asarray(inputs["Wv"], dtype=np.float32)
    Wo = np.asarray(inputs["Wo"], dtype=np.float32)
    bo = np.asarray(inputs["bo"], dtype=np.float32)
    assert not np.any(bo != 0), "nonzero output bias not supported"

    Aq, Bq = _rope_tables(np.asarray(inputs["qn_w"], np.float32),
                          np.asarray(inputs["qn_b"], np.float32), L, D ** -0.5)
    Ak, Bk = _rope_tables(np.asarray(inputs["kn_w"], np.float32),
                          np.asarray(inputs["kn_b"], np.float32), L, 1.0)

    wqT = np.ascontiguousarray(_fold_mean(Wq).T).astype(BF16_NP)
    wkT = np.ascontiguousarray(_fold_mean(Wk).T).astype(BF16_NP)
    wvT = np.ascontiguousarray(Wv.T.astype(np.float64)).astype(BF16_NP)
    # o_proj pair-packed: WoT rows grouped (pair, 2 heads x 64)
    woP = np.ascontiguousarray(
        Wo.T.astype(np.float64).reshape(NPR, P, C)).astype(BF16_NP)

    if "nc" not in _NC_CACHE:
        _NC_CACHE["nc"] = _build_nc()
    nc = _NC_CACHE["nc"]

    in_maps = []
    for c in range(8):
        b_, half = c // 2, c % 2
        xT = np.ascontiguousarray(x[b_].T.astype(np.float64)).astype(BF16_NP)
        in_maps.append({
            "xT": xT,
            "xqT": np.ascontiguousarray(xT[:, half * LQ:(half + 1) * LQ]),
            "wqT": wqT, "wkT": wkT, "wvT": wvT, "woP": woP,
            "aq": np.ascontiguousarray(Aq[half * LQ:(half + 1) * LQ]),
            "bq": np.ascontiguousarray(Bq[half * LQ:(half + 1) * LQ]),
            "ak": Ak, "bk": Bk,
        })

    res = run_bass_kernel_spmd(nc, in_maps, core_ids=list(range(8)))
    out = np.empty((B, L, C), dtype=np.float32)
    for c in range(8):
        b_, half = c // 2, c % 2
        out[b_, half * LQ:(half + 1) * LQ] = res.results[c]["out"]
    return out


# revision 19
# speedup vs baseline: 2.0059x; 1.3531x over previous
"""MultiHeadAttention (qk-LayerNorm + RoPE) Trainium2 kernel, 8 NeuronCores.

Sharding: batch (4) x query-half (2x512 tokens), collective-free. Core c
handles batch c//2, query rows (c%2)*512 .. +512, ALL 16 heads. K/V
projections for the full 1024-token sequence are duplicated within each
batch pair; in exchange there are no collectives at all (the multi-device
rendezvous barrier plus ReduceScatter cost ~120us in the pair-parallel
variant). Each core writes its 512 output rows directly; the host
concatenates.

All matmuls run in bf16 (1 col/cycle on the PE, half the DMA/SBUF of
f32r). LayerNorm means are folded into Wq/Wk on the host (per-head
row-block mean subtraction makes projections zero-mean), so only the
variance is computed on device. D^-0.5 is folded into the q rope tables.
Scores are computed transposed ([s, t]) so the softmax denominator comes
free from a ones column appended to V; normalization uses one
reciprocal_approx_fast over head-stacked sums and a DMA partition
broadcast. o_proj packs head pairs (2x64 rows) for full 128-deep
contraction.
"""
import sys

for _p in ("/opt/trn_rl_repo", "/root/.axon_site", "/root/.axon_site/_ro/trn_rl_repo",
           "/root/.axon_site/_ro/pypackages"):
    if _p not in sys.path:
        sys.path.append(_p)

import numpy as np
import ml_dtypes

import concourse.bass as bass
import concourse.tile as tile
from concourse import bacc, mybir
from concourse.bass_utils import run_bass_kernel_spmd
from concourse.masks import make_identity

BF16_NP = ml_dtypes.bfloat16
F32 = mybir.dt.float32
BF16 = mybir.dt.bfloat16
P = 128
B, L, C, H, D = 4, 1024, 1024, 16, 64
LQ = L // 2          # query rows per core
NTQ = LQ // P        # 4 query token tiles
NT = L // P          # 8 key token tiles
NCK = C // P         # 8 contraction tiles
NPR = H // 2         # 8 head pairs
THETA = 50000.0
EPS = 1e-5

_NC_CACHE = {}


def _build_nc():
    nc = bacc.Bacc("TRN2", target_bir_lowering=False, debug=False, num_devices=8)

    xT_d = nc.dram_tensor("xT", [C, L], BF16, kind="ExternalInput")
    xqT_d = nc.dram_tensor("xqT", [C, LQ], BF16, kind="ExternalInput")
    wqT_d = nc.dram_tensor("wqT", [C, C], BF16, kind="ExternalInput")
    wkT_d = nc.dram_tensor("wkT", [C, C], BF16, kind="ExternalInput")
    wvT_d = nc.dram_tensor("wvT", [C, C], BF16, kind="ExternalInput")
    woP_d = nc.dram_tensor("woP", [NPR, P, C], BF16, kind="ExternalInput")
    aq_d = nc.dram_tensor("aq", [LQ, D], BF16, kind="ExternalInput")
    bq_d = nc.dram_tensor("bq", [LQ, D], BF16, kind="ExternalInput")
    ak_d = nc.dram_tensor("ak", [L, D], BF16, kind="ExternalInput")
    bk_d = nc.dram_tensor("bk", [L, D], BF16, kind="ExternalInput")
    out_d = nc.dram_tensor("out", [LQ, C], F32, kind="ExternalOutput")

    with tile.TileContext(nc) as tc:
        with (
            tc.tile_pool(name="const", bufs=1) as constp,
            tc.tile_pool(name="w", bufs=1) as wpool,
            tc.tile_pool(name="big", bufs=1) as bigp,
            tc.tile_pool(name="xt", bufs=2) as xtp,
            tc.tile_pool(name="stg", bufs=2) as stgp,
            tc.tile_pool(name="stat", bufs=2) as statp,
            tc.tile_pool(name="exp", bufs=2) as expp,
            tc.tile_pool(name="fin", bufs=2) as finp,
            tc.tile_pool(name="dram", bufs=1, space="DRAM") as dramp,
        ):
            ident = constp.tile([P, P], BF16)
            make_identity(nc, ident)
            eps_t = constp.tile([P, 1], F32)
            nc.vector.memset(eps_t[:], EPS)
            ones_row = constp.tile([1, D], BF16)
            nc.vector.memset(ones_row[:], 1.0)
            one_f32 = constp.tile([1, 1], F32)
            nc.vector.memset(one_f32[:], 1.0)

            aq_t = constp.tile([P, NTQ, D], BF16)
            nc.sync.dma_start(aq_t[:], aq_d.ap().rearrange("(t p) d -> p t d", p=P))
            bq_t = constp.tile([P, NTQ, D], BF16)
            nc.sync.dma_start(bq_t[:], bq_d.ap().rearrange("(t p) d -> p t d", p=P))
            ak_t = constp.tile([P, NT, D], BF16)
            nc.sync.dma_start(ak_t[:], ak_d.ap().rearrange("(t p) d -> p t d", p=P))
            bk_t = constp.tile([P, NT, D], BF16)
            nc.sync.dma_start(bk_t[:], bk_d.ap().rearrange("(t p) d -> p t d", p=P))

            # per-ck weight tiles; q weights first so phase Q starts early
            wq_t, wk_t, wv_t = [], [], []
            for lst, nm, d_ in ((wq_t, "wq", wqT_d), (wk_t, "wk", wkT_d),
                                (wv_t, "wv", wvT_d)):
                for ck in range(NCK):
                    t_ = wpool.tile([P, C], BF16, tag=f"{nm}{ck}", name=f"{nm}{ck}")
                    nc.sync.dma_start(
                        t_[:], d_.ap().rearrange("(k p) o -> p k o", p=P)[:, ck, :])
                    lst.append(t_)
            wo_l = []
            for pr in range(NPR):
                t_ = wpool.tile([P, C], BF16, tag=f"wo{pr}", name=f"wo{pr}")
                nc.sync.dma_start(t_[:], woP_d.ap()[pr])
                wo_l.append(t_)

            # v with a ones column appended per head: [s, j, h, 65]
            v_sb = bigp.tile([P, NT, H, D + 1], BF16)
            nc.gpsimd.memset(v_sb[:, :, :, D:D + 1], 1.0)

            qT_pack = bigp.tile([P, NPR, LQ], BF16)
            kT_pack = bigp.tile([P, NPR, L], BF16)
            ctxT = bigp.tile([P, NPR, LQ], BF16)

            def ln_rope(ps, a_t, b_t, ti, dst_pack, pst_pool):
                """psum [t,1024] f32 -> LN(var-only)+rope -> transpose into
                dst_pack[:, pr, ti*128:...]. Returns nothing."""
                qs = stgp.tile([P, H, D], BF16, tag="stg")
                nc.scalar.copy(qs[:], ps[:].rearrange("p (h d) -> p h d", d=D))
                sq = stgp.tile([P, H, D], BF16, tag="sq")
                nc.vector.tensor_mul(sq[:], qs[:], qs[:])
                ss = statp.tile([P, H], F32, tag="ss")
                nc.vector.reduce_sum(ss[:], sq[:], axis=mybir.AxisListType.X)
                std = statp.tile([P, H], F32, tag="std")
                nc.scalar.activation(std[:], ss[:], mybir.ActivationFunctionType.Sqrt,
                                     bias=eps_t[:], scale=1.0 / D)
                inv = statp.tile([P, H], F32, tag="inv")
                nc.vector.reciprocal(inv[:], std[:])
                invb = statp.tile([P, H], BF16, tag="invb")
                nc.vector.tensor_copy(invb[:], inv[:])

                a_b = a_t[:, ti, :].rearrange("p d -> p () d").to_broadcast((P, H, D))
                r = stgp.tile([P, H, D], BF16, tag="r")
                nc.vector.tensor_mul(r[:], qs[:], a_b)
                r2 = stgp.tile([P, H, D], BF16, tag="r2")
                h_ = D // 2
                nc.vector.tensor_mul(
                    r2[:, :, 0:h_], qs[:, :, h_:D],
                    b_t[:, ti, 0:h_].rearrange("p d -> p () d").to_broadcast((P, H, h_)))
                nc.vector.tensor_mul(
                    r2[:, :, h_:D], qs[:, :, 0:h_],
                    b_t[:, ti, h_:D].rearrange("p d -> p () d").to_broadcast((P, H, h_)))
                nc.vector.tensor_add(r[:], r[:], r2[:])
                nc.vector.tensor_mul(
                    r[:], r[:],
                    invb[:].rearrange("p h -> p h ()").to_broadcast((P, H, D)))

                for grp in range(2):
                    pst = pst_pool.tile([P, 4, P], BF16, tag="pst")
                    for q4 in range(4):
                        pr = grp * 4 + q4
                        nc.tensor.transpose(
                            pst[:, q4, :],
                            r[:, 2 * pr:2 * pr + 2, :].rearrange("p h d -> p (h d)"),
                            ident[:])
                    nc.vector.tensor_copy(
                        dst_pack[:, 4 * grp:4 * grp + 4, bass.ts(ti, P)], pst[:])

            # ---------------- Phase Q: q projection + LN + RoPE -------------
            with tc.tile_pool(name="psq", bufs=2, space="PSUM") as psqp, \
                 tc.tile_pool(name="pstq", bufs=2, space="PSUM") as pstqp:
                for ti in range(NTQ):
                    xt = xtp.tile([P, NCK, P], BF16, tag="xq")
                    nc.scalar.dma_start(
                        xt[:],
                        xqT_d.ap().rearrange("(k p) t -> p k t", p=P)[:, :, bass.ts(ti, P)])
                    psq = psqp.tile([P, C], F32)
                    for ch in range(2):
                        for ck in range(NCK):
                            nc.tensor.matmul(psq[:, bass.ts(ch, 512)],
                                             xt[:, ck, :],
                                             wq_t[ck][:, bass.ts(ch, 512)],
                                             start=(ck == 0), stop=(ck == NCK - 1))
                    ln_rope(psq, aq_t, bq_t, ti, qT_pack, pstqp)

            # ---------------- Phase KV: k/v projection + LN + RoPE ----------
            with tc.tile_pool(name="psk", bufs=2, space="PSUM") as pskp, \
                 tc.tile_pool(name="psv", bufs=1, space="PSUM") as psvp, \
                 tc.tile_pool(name="pstk", bufs=2, space="PSUM") as pstkp:
                for ti in range(NT):
                    xt = xtp.tile([P, NCK, P], BF16, tag="xk")
                    nc.scalar.dma_start(
                        xt[:],
                        xT_d.ap().rearrange("(k p) t -> p k t", p=P)[:, :, bass.ts(ti, P)])
                    psk = pskp.tile([P, C], F32)
                    psv = psvp.tile([P, C], F32)
                    for ps_, w_ in ((psk, wk_t), (psv, wv_t)):
                        for ch in range(2):
                            for ck in range(NCK):
                                nc.tensor.matmul(ps_[:, bass.ts(ch, 512)],
                                                 xt[:, ck, :],
                                                 w_[ck][:, bass.ts(ch, 512)],
                                                 start=(ck == 0), stop=(ck == NCK - 1))
                    nc.scalar.copy(
                        v_sb[:, ti, :, 0:D],
                        psv[:].rearrange("p (h d) -> p h d", d=D))
                    ln_rope(psk, ak_t, bk_t, ti, kT_pack, pstkp)

            # ---------------- Phase ATT: attention, head pair per round -----
            with tc.tile_pool(name="pss", bufs=2, space="PSUM") as pssp, \
                 tc.tile_pool(name="psc", bufs=1, space="PSUM") as pscp, \
                 tc.tile_pool(name="psst", bufs=1, space="PSUM") as psstp, \
                 tc.tile_pool(name="psrb", bufs=1, space="PSUM") as psrbp:
                for pr in range(NPR):
                    psc = [pscp.tile([D + 1, LQ], F32, tag=f"c{i}",
                                     name=f"psc{pr}_{i}") for i in range(2)]
                    for j in range(NT):
                        pss = pssp.tile([P, 2, LQ], F32, tag="pss")
                        for i in range(2):
                            lo = i * D
                            nc.tensor.matmul(
                                pss[:, i, :],
                                kT_pack[lo:lo + D, pr, bass.ts(j, P)],
                                qT_pack[lo:lo + D, pr, :],
                                start=True, stop=True)
                        expT = expp.tile([P, 2, LQ], BF16, tag="expT")
                        nc.scalar.activation(expT[:], pss[:],
                                             mybir.ActivationFunctionType.Exp)
                        for i in range(2):
                            nc.tensor.matmul(
                                psc[i][:],
                                v_sb[:, j, 2 * pr + i, :],
                                expT[:, i, :],
                                start=(j == 0), stop=(j == NT - 1))
                    # drain pair: unnormalized ctx (2 heads packed on 128p)
                    # and sums; reciprocal runs 128-way via transposed layout
                    smq = finp.tile([1, 2, LQ], F32, tag="smq")
                    for i in range(2):
                        lo = i * D
                        nc.vector.tensor_copy(
                            ctxT[lo:lo + D, pr, :], psc[i][0:D, :])
                        nc.vector.tensor_copy(smq[0:1, i, :], psc[i][D:D + 1, :])
                    sumsT = psstp.tile([P, 2, 4], F32, tag="sumsT")
                    for i in range(2):
                        for c in range(4):
                            nc.tensor.transpose(
                                sumsT[:, i, c:c + 1],
                                smq[0:1, i, bass.ts(c, P)], one_f32[:])
                    rbT = finp.tile([P, 2, 4], BF16, tag="rbT")
                    with nc.allow_low_precision(reason="softmax recip bf16"):
                        nc.vector.reciprocal(rbT[:], sumsT[:])
                    rbrow_ps = psrbp.tile([1, 2, LQ], BF16, tag="rbrow")
                    for i in range(2):
                        for c in range(4):
                            nc.tensor.transpose(
                                rbrow_ps[0:1, i, bass.ts(c, P)],
                                rbT[:, i, c:c + 1], ident[:])
                    rbrow = finp.tile([1, 2, LQ], BF16, tag="rbrow_sb")
                    nc.vector.tensor_copy(rbrow[:], rbrow_ps[:])
                    # broadcast 1/sums across 64 partitions via K=1 matmul
                    # (reuses freed psc bank slots), then normalize in place
                    for i in range(2):
                        lo = i * D
                        rb_ps = pscp.tile([D, LQ], F32, tag=f"c{i}",
                                          name=f"rbps{pr}_{i}")
                        nc.tensor.matmul(rb_ps[:], ones_row[:],
                                         rbrow[0:1, i, :], start=True, stop=True)
                        nc.vector.tensor_mul(ctxT[lo:lo + D, pr, :],
                                             ctxT[lo:lo + D, pr, :],
                                             rb_ps[:])

            # ---------------- Phase O: output projection --------------------
            with tc.tile_pool(name="pso", bufs=2, space="PSUM") as psop:
                for ti in range(NTQ):
                    pso = psop.tile([P, C], F32)
                    for pr in range(NPR):
                        for ch in range(2):
                            nc.tensor.matmul(
                                pso[:, bass.ts(ch, 512)],
                                ctxT[:, pr, bass.ts(ti, P)],
                                wo_l[pr][:, bass.ts(ch, 512)],
                                start=(pr == 0), stop=(pr == NPR - 1))
                    out_sb = finp.tile([P, C], F32, tag="out")
                    nc.vector.tensor_copy(out_sb[:], pso[:])
                    nc.sync.dma_start(out_d.ap()[bass.ts(ti, P), :], out_sb[:])

    nc.compile()
    return nc


def _rope_tables(w, b, length, scale):
    """A[t,d], B[t,d] with rotate-half sign and LN weight folded in."""
    inv_freq = 1.0 / THETA ** (np.arange(0, D, 2, dtype=np.float64) / D)
    freqs = np.arange(length, dtype=np.float64)[:, None] * inv_freq[None, :]
    freqs = np.concatenate([freqs, freqs], axis=1)
    cos, sin = np.cos(freqs), np.sin(freqs)
    w = w.astype(np.float64)
    w_rot = np.concatenate([w[D // 2:], w[:D // 2]])
    sgn = np.concatenate([-np.ones(D // 2), np.ones(D // 2)])
    A = (cos * w[None, :] * scale).astype(BF16_NP)
    Bt = (sin * w_rot[None, :] * sgn[None, :] * scale).astype(BF16_NP)
    if np.any(b != 0):
        raise NotImplementedError("nonzero qk-norm bias not supported")
    return A, Bt


def _fold_mean(W):
    """Remove per-head row-block mean: projections become zero-mean."""
    W = W.astype(np.float64).copy()
    for h in range(H):
        W[h * D:(h + 1) * D, :] -= W[h * D:(h + 1) * D, :].mean(0, keepdims=True)
    return W


def kernel(**inputs):
    x = np.asarray(inputs["q"], dtype=np.float32)
    Wq = np.asarray(inputs["Wq"], dtype=np.float32)
    Wk = np.asarray(inputs["Wk"], dtype=np.float32)
    Wv = np.asarray(inputs["Wv"], dtype=np.float32)
    Wo = np.asarray(inputs["Wo"], dtype=np.float32)
    bo = np.asarray(inputs["bo"], dtype=np.float32)
    assert not np.any(bo != 0), "nonzero output bias not supported"

    Aq, Bq = _rope_tables(np.asarray(inputs["qn_w"], np.float32),
                          np.asarray(inputs["qn_b"], np.float32), L, D ** -0.5)
    Ak, Bk = _rope_tables(np.asarray(inputs["kn_w"], np.float32),
                          np.asarray(inputs["kn_b"], np.float32), L, 1.0)

    wqT = np.ascontiguousarray(_fold_mean(Wq).T).astype(BF16_NP)
    wkT = np.ascontiguousarray(_fold_mean(Wk).T).astype(BF16_NP)
    wvT = np.ascontiguousarray(Wv.T.astype(np.float64)).astype(BF16_NP)
    # o_proj pair-packed: WoT rows grouped (pair, 2 heads x 64)
    woP = np.ascontiguousarray(
        Wo.T.astype(np.float64).reshape(NPR, P, C)).astype(BF16_NP)

    if "nc" not in _NC_CACHE:
        _NC_CACHE["nc"] = _build_nc()
    nc = _NC_CACHE["nc"]

    in_maps = []
    for c in range(8):
        b_, half = c // 2, c % 2
        xT = np.ascontiguousarray(x[b_].T.astype(np.float64)).astype(BF16_NP)
        in_maps.append({
            "xT": xT,
            "xqT": np.ascontiguousarray(xT[:, half * LQ:(half + 1) * LQ]),
            "wqT": wqT, "wkT": wkT, "wvT": wvT, "woP": woP,
            "aq": np.ascontiguousarray(Aq[half * LQ:(half + 1) * LQ]),
            "bq": np.ascontiguousarray(Bq[half * LQ:(half + 1) * LQ]),
            "ak": Ak, "bk": Bk,
        })

    res = run_bass_kernel_spmd(nc, in_maps, core_ids=list(range(8)))
    out = np.empty((B, L, C), dtype=np.float32)
    for c in range(8):
        b_, half = c // 2, c % 2
        out[b_, half * LQ:(half + 1) * LQ] = res.results[c]["out"]
    return out


# revision 21
# speedup vs baseline: 2.2563x; 1.1248x over previous
"""MultiHeadAttention (qk-LayerNorm + RoPE) Trainium2 kernel, 8 NeuronCores.

Sharding: batch (4) x query-half (2x512 tokens), collective-free. Core c
handles batch c//2, query rows (c%2)*512 .. +512, ALL 16 heads. K/V
projections for the full 1024-token sequence are duplicated within each
batch pair; in exchange there are no collectives at all (the multi-device
rendezvous barrier plus ReduceScatter cost ~120us in the pair-parallel
variant). Each core writes its 512 output rows directly; the host
concatenates.

All matmuls run in bf16 (1 col/cycle on the PE, half the DMA/SBUF of
f32r). LayerNorm means are folded into Wq/Wk on the host (per-head
row-block mean subtraction makes projections zero-mean), so only the
variance is computed on device. D^-0.5 is folded into the q rope tables.
Scores are computed transposed ([s, t]) so the softmax denominator comes
free from a ones column appended to V; normalization uses one
reciprocal_approx_fast over head-stacked sums and a DMA partition
broadcast. o_proj packs head pairs (2x64 rows) for full 128-deep
contraction.
"""
import sys

for _p in ("/opt/trn_rl_repo", "/root/.axon_site", "/root/.axon_site/_ro/trn_rl_repo",
           "/root/.axon_site/_ro/pypackages"):
    if _p not in sys.path:
        sys.path.append(_p)

import numpy as np
import ml_dtypes

import concourse.bass as bass
import concourse.tile as tile
from concourse import bacc, mybir
from concourse.bass_utils import run_bass_kernel_spmd
from concourse.masks import make_identity

BF16_NP = ml_dtypes.bfloat16
F32 = mybir.dt.float32
BF16 = mybir.dt.bfloat16
P = 128
B, L, C, H, D = 4, 1024, 1024, 16, 64
LQ = L // 2          # query rows per core
NTQ = LQ // P        # 4 query token tiles
NT = L // P          # 8 key token tiles
NCK = C // P         # 8 contraction tiles
NPR = H // 2         # 8 head pairs
THETA = 50000.0
EPS = 1e-5

_NC_CACHE = {}


def _build_nc():
    nc = bacc.Bacc("TRN2", target_bir_lowering=False, debug=False, num_devices=8)

    xT_d = nc.dram_tensor("xT", [C, L], BF16, kind="ExternalInput")
    xqT_d = nc.dram_tensor("xqT", [C, LQ], BF16, kind="ExternalInput")
    wqT_d = nc.dram_tensor("wqT", [C, C], BF16, kind="ExternalInput")
    wkT_d = nc.dram_tensor("wkT", [C, C], BF16, kind="ExternalInput")
    wvT_d = nc.dram_tensor("wvT", [C, C], BF16, kind="ExternalInput")
    woP_d = nc.dram_tensor("woP", [NPR, P, C], BF16, kind="ExternalInput")
    aq_d = nc.dram_tensor("aq", [LQ, D], BF16, kind="ExternalInput")
    bq_d = nc.dram_tensor("bq", [LQ, D], BF16, kind="ExternalInput")
    ak_d = nc.dram_tensor("ak", [L, D], BF16, kind="ExternalInput")
    bk_d = nc.dram_tensor("bk", [L, D], BF16, kind="ExternalInput")
    out_d = nc.dram_tensor("out", [LQ, C], F32, kind="ExternalOutput")

    with tile.TileContext(nc) as tc:
        with (
            tc.tile_pool(name="const", bufs=1) as constp,
            tc.tile_pool(name="w", bufs=1) as wpool,
            tc.tile_pool(name="big", bufs=1) as bigp,
            tc.tile_pool(name="xt", bufs=2) as xtp,
            tc.tile_pool(name="stg", bufs=2) as stgp,
            tc.tile_pool(name="stat", bufs=2) as statp,
            tc.tile_pool(name="exp", bufs=2) as expp,
            tc.tile_pool(name="fin", bufs=2) as finp,
            tc.tile_pool(name="dram", bufs=1, space="DRAM") as dramp,
        ):
            ident = constp.tile([P, P], BF16)
            make_identity(nc, ident)
            eps_t = constp.tile([P, 1], F32)
            nc.vector.memset(eps_t[:], EPS)
            ones_row = constp.tile([1, D], BF16)
            nc.vector.memset(ones_row[:], 1.0)
            one_f32 = constp.tile([1, 1], F32)
            nc.vector.memset(one_f32[:], 1.0)

            aq_t = constp.tile([P, NTQ, D], BF16)
            nc.sync.dma_start(aq_t[:], aq_d.ap().rearrange("(t p) d -> p t d", p=P))
            bq_t = constp.tile([P, NTQ, D], BF16)
            nc.sync.dma_start(bq_t[:], bq_d.ap().rearrange("(t p) d -> p t d", p=P))
            ak_t = constp.tile([P, NT, D], BF16)
            nc.sync.dma_start(ak_t[:], ak_d.ap().rearrange("(t p) d -> p t d", p=P))
            bk_t = constp.tile([P, NT, D], BF16)
            nc.sync.dma_start(bk_t[:], bk_d.ap().rearrange("(t p) d -> p t d", p=P))

            # per-ck weight tiles; q weights first so phase Q starts early
            wq_t, wk_t, wv_t = [], [], []
            for lst, nm, d_ in ((wq_t, "wq", wqT_d), (wk_t, "wk", wkT_d),
                                (wv_t, "wv", wvT_d)):
                for ck in range(NCK):
                    t_ = wpool.tile([P, C], BF16, tag=f"{nm}{ck}", name=f"{nm}{ck}")
                    nc.sync.dma_start(
                        t_[:], d_.ap().rearrange("(k p) o -> p k o", p=P)[:, ck, :])
                    lst.append(t_)
            wo_l = []
            for pr in range(NPR):
                t_ = wpool.tile([P, C], BF16, tag=f"wo{pr}", name=f"wo{pr}")
                nc.sync.dma_start(t_[:], woP_d.ap()[pr])
                wo_l.append(t_)

            # v with a ones column appended per head: [s, j, h, 65]
            v_sb = bigp.tile([P, NT, H, D + 1], BF16)
            nc.gpsimd.memset(v_sb[:, :, :, D:D + 1], 1.0)

            qT_pack = bigp.tile([P, NPR, LQ], BF16)
            kT_pack = bigp.tile([P, NPR, L], BF16)
            ctxT = bigp.tile([P, NPR, LQ], BF16)

            def ln_rope(ps, a_t, b_t, ti, dst_pack, pst_pool):
                """psum [t,1024] f32 -> LN(var-only)+rope -> transpose into
                dst_pack[:, pr, ti*128:...]. Returns nothing."""
                qs = stgp.tile([P, H, D], BF16, tag="stg")
                nc.scalar.copy(qs[:], ps[:].rearrange("p (h d) -> p h d", d=D))
                sq = stgp.tile([P, H, D], BF16, tag="sq")
                nc.scalar.square(sq[:], qs[:])
                ss = statp.tile([P, H], F32, tag="ss")
                nc.vector.reduce_sum(ss[:], sq[:], axis=mybir.AxisListType.X)
                std = statp.tile([P, H], F32, tag="std")
                nc.scalar.activation(std[:], ss[:], mybir.ActivationFunctionType.Sqrt,
                                     bias=eps_t[:], scale=1.0 / D)
                inv = statp.tile([P, H], F32, tag="inv")
                nc.vector.reciprocal(inv[:], std[:])
                invb = statp.tile([P, H], BF16, tag="invb")
                nc.vector.tensor_copy(invb[:], inv[:])

                a_b = a_t[:, ti, :].rearrange("p d -> p () d").to_broadcast((P, H, D))
                r = stgp.tile([P, H, D], BF16, tag="r")
                nc.vector.tensor_mul(r[:], qs[:], a_b)
                r2 = stgp.tile([P, H, D], BF16, tag="r2")
                h_ = D // 2
                nc.vector.tensor_mul(
                    r2[:, :, 0:h_], qs[:, :, h_:D],
                    b_t[:, ti, 0:h_].rearrange("p d -> p () d").to_broadcast((P, H, h_)))
                nc.vector.tensor_mul(
                    r2[:, :, h_:D], qs[:, :, 0:h_],
                    b_t[:, ti, h_:D].rearrange("p d -> p () d").to_broadcast((P, H, h_)))
                nc.vector.tensor_add(r[:], r[:], r2[:])
                nc.vector.tensor_mul(
                    r[:], r[:],
                    invb[:].rearrange("p h -> p h ()").to_broadcast((P, H, D)))

                for grp in range(2):
                    pst = pst_pool.tile([P, 4, P], BF16, tag="pst")
                    for q4 in range(4):
                        pr = grp * 4 + q4
                        nc.tensor.transpose(
                            pst[:, q4, :],
                            r[:, 2 * pr:2 * pr + 2, :].rearrange("p h d -> p (h d)"),
                            ident[:])
                    nc.vector.tensor_copy(
                        dst_pack[:, 4 * grp:4 * grp + 4, bass.ts(ti, P)], pst[:])

            # ---------------- Phase Q: q projection + LN + RoPE -------------
            with tc.tile_pool(name="psq", bufs=2, space="PSUM") as psqp, \
                 tc.tile_pool(name="pstq", bufs=2, space="PSUM") as pstqp:
                for ti in range(NTQ):
                    xt = xtp.tile([P, NCK, P], BF16, tag="xq")
                    nc.scalar.dma_start(
                        xt[:],
                        xqT_d.ap().rearrange("(k p) t -> p k t", p=P)[:, :, bass.ts(ti, P)])
                    psq = psqp.tile([P, C], F32)
                    for ch in range(2):
                        for ck in range(NCK):
                            nc.tensor.matmul(psq[:, bass.ts(ch, 512)],
                                             xt[:, ck, :],
                                             wq_t[ck][:, bass.ts(ch, 512)],
                                             start=(ck == 0), stop=(ck == NCK - 1))
                    ln_rope(psq, aq_t, bq_t, ti, qT_pack, pstqp)

            # ---------------- Phase KV: k/v projection + LN + RoPE ----------
            with tc.tile_pool(name="psk", bufs=2, space="PSUM") as pskp, \
                 tc.tile_pool(name="psv", bufs=1, space="PSUM") as psvp, \
                 tc.tile_pool(name="pstk", bufs=2, space="PSUM") as pstkp:
                for ti in range(NT):
                    xt = xtp.tile([P, NCK, P], BF16, tag="xk")
                    nc.scalar.dma_start(
                        xt[:],
                        xT_d.ap().rearrange("(k p) t -> p k t", p=P)[:, :, bass.ts(ti, P)])
                    psk = pskp.tile([P, C], F32)
                    psv = psvp.tile([P, C], F32)
                    for ps_, w_ in ((psk, wk_t), (psv, wv_t)):
                        for ch in range(2):
                            for ck in range(NCK):
                                nc.tensor.matmul(ps_[:, bass.ts(ch, 512)],
                                                 xt[:, ck, :],
                                                 w_[ck][:, bass.ts(ch, 512)],
                                                 start=(ck == 0), stop=(ck == NCK - 1))
                    nc.scalar.copy(
                        v_sb[:, ti, :, 0:D],
                        psv[:].rearrange("p (h d) -> p h d", d=D))
                    ln_rope(psk, ak_t, bk_t, ti, kT_pack, pstkp)

            # ---------------- Phase ATT: attention, head pair per round -----
            with tc.tile_pool(name="pss", bufs=2, space="PSUM") as pssp, \
                 tc.tile_pool(name="psc", bufs=1, space="PSUM") as pscp, \
                 tc.tile_pool(name="psst", bufs=1, space="PSUM") as psstp, \
                 tc.tile_pool(name="psrb", bufs=1, space="PSUM") as psrbp:
                for pr in range(NPR):
                    psc = [pscp.tile([D + 1, LQ], F32, tag=f"c{i}",
                                     name=f"psc{pr}_{i}") for i in range(2)]
                    for j in range(NT):
                        pss = pssp.tile([P, 2, LQ], F32, tag="pss")
                        for i in range(2):
                            lo = i * D
                            nc.tensor.matmul(
                                pss[:, i, :],
                                kT_pack[lo:lo + D, pr, bass.ts(j, P)],
                                qT_pack[lo:lo + D, pr, :],
                                start=True, stop=True)
                        expT = expp.tile([P, 2, LQ], BF16, tag="expT")
                        nc.scalar.activation(expT[:], pss[:],
                                             mybir.ActivationFunctionType.Exp)
                        for i in range(2):
                            nc.tensor.matmul(
                                psc[i][:],
                                v_sb[:, j, 2 * pr + i, :],
                                expT[:, i, :],
                                start=(j == 0), stop=(j == NT - 1))
                    # drain pair: unnormalized ctx (2 heads packed on 128p)
                    # and sums; reciprocal runs 128-way via transposed layout
                    smq = finp.tile([1, 2, LQ], F32, tag="smq")
                    for i in range(2):
                        lo = i * D
                        nc.vector.tensor_copy(
                            ctxT[lo:lo + D, pr, :], psc[i][0:D, :])
                        nc.vector.tensor_copy(smq[0:1, i, :], psc[i][D:D + 1, :])
                    sumsT = psstp.tile([P, 2, 4], F32, tag="sumsT")
                    for i in range(2):
                        for c in range(4):
                            nc.tensor.transpose(
                                sumsT[:, i, c:c + 1],
                                smq[0:1, i, bass.ts(c, P)], one_f32[:])
                    rbT = finp.tile([P, 2, 4], BF16, tag="rbT")
                    with nc.allow_low_precision(reason="softmax recip bf16"):
                        nc.vector.reciprocal(rbT[:], sumsT[:])
                    rbrow_ps = psrbp.tile([1, 2, LQ], BF16, tag="rbrow")
                    for i in range(2):
                        for c in range(4):
                            nc.tensor.transpose(
                                rbrow_ps[0:1, i, bass.ts(c, P)],
                                rbT[:, i, c:c + 1], ident[:])
                    rbrow = finp.tile([1, 2, LQ], BF16, tag="rbrow_sb")
                    nc.vector.tensor_copy(rbrow[:], rbrow_ps[:])
                    # broadcast 1/sums across 64 partitions via K=1 matmul.
                    # rb_ps shares the rbrow bank (sequential use) so the psc
                    # banks free right after the drain copies — the next
                    # pair's ctx accumulation starts without waiting on the
                    # normalize chain.
                    for i in range(2):
                        lo = i * D
                        rb_ps = psrbp.tile([D, LQ], F32, tag="rbrow",
                                           name=f"rbps{pr}_{i}")
                        nc.tensor.matmul(rb_ps[:], ones_row[:],
                                         rbrow[0:1, i, :], start=True, stop=True)
                        nc.vector.tensor_mul(ctxT[lo:lo + D, pr, :],
                                             ctxT[lo:lo + D, pr, :],
                                             rb_ps[:])

            # ---------------- Phase O: output projection --------------------
            with tc.tile_pool(name="pso", bufs=2, space="PSUM") as psop:
                for ti in range(NTQ):
                    pso = psop.tile([P, C], F32)
                    for pr in range(NPR):
                        for ch in range(2):
                            nc.tensor.matmul(
                                pso[:, bass.ts(ch, 512)],
                                ctxT[:, pr, bass.ts(ti, P)],
                                wo_l[pr][:, bass.ts(ch, 512)],
                                start=(pr == 0), stop=(pr == NPR - 1))
                    out_sb = finp.tile([P, C], F32, tag="out")
                    nc.vector.tensor_copy(out_sb[:], pso[:])
                    nc.sync.dma_start(out_d.ap()[bass.ts(ti, P), :], out_sb[:])

    nc.compile()
    return nc


def _rope_tables(w, b, length, scale):
    """A[t,d], B[t,d] with rotate-half sign and LN weight folded in."""
    inv_freq = 1.0 / THETA ** (np.arange(0, D, 2, dtype=np.float64) / D)
    freqs = np.arange(length, dtype=np.float64)[:, None] * inv_freq[None, :]
    freqs = np.concatenate([freqs, freqs], axis=1)
    cos, sin = np.cos(freqs), np.sin(freqs)
    w = w.astype(np.float64)
    w_rot = np.concatenate([w[D // 2:], w[:D // 2]])
    sgn = np.concatenate([-np.ones(D // 2), np.ones(D // 2)])
    A = (cos * w[None, :] * scale).astype(BF16_NP)
    Bt = (sin * w_rot[None, :] * sgn[None, :] * scale).astype(BF16_NP)
    if np.any(b != 0):
        raise NotImplementedError("nonzero qk-norm bias not supported")
    return A, Bt


def _fold_mean(W):
    """Remove per-head row-block mean: projections become zero-mean."""
    W = W.astype(np.float64).copy()
    for h in range(H):
        W[h * D:(h + 1) * D, :] -= W[h * D:(h + 1) * D, :].mean(0, keepdims=True)
    return W


def kernel(**inputs):
    x = np.asarray(inputs["q"], dtype=np.float32)
    Wq = np.asarray(inputs["Wq"], dtype=np.float32)
    Wk = np.asarray(inputs["Wk"], dtype=np.float32)
    Wv = np.asarray(inputs["Wv"], dtype=np.float32)
    Wo = np.asarray(inputs["Wo"], dtype=np.float32)
    bo = np.asarray(inputs["bo"], dtype=np.float32)
    assert not np.any(bo != 0), "nonzero output bias not supported"

    Aq, Bq = _rope_tables(np.asarray(inputs["qn_w"], np.float32),
                          np.asarray(inputs["qn_b"], np.float32), L, D ** -0.5)
    Ak, Bk = _rope_tables(np.asarray(inputs["kn_w"], np.float32),
                          np.asarray(inputs["kn_b"], np.float32), L, 1.0)

    wqT = np.ascontiguousarray(_fold_mean(Wq).T).astype(BF16_NP)
    wkT = np.ascontiguousarray(_fold_mean(Wk).T).astype(BF16_NP)
    wvT = np.ascontiguousarray(Wv.T.astype(np.float64)).astype(BF16_NP)
    # o_proj pair-packed: WoT rows grouped (pair, 2 heads x 64)
    woP = np.ascontiguousarray(
        Wo.T.astype(np.float64).reshape(NPR, P, C)).astype(BF16_NP)

    if "nc" not in _NC_CACHE:
        _NC_CACHE["nc"] = _build_nc()
    nc = _NC_CACHE["nc"]

    in_maps = []
    for c in range(8):
        b_, half = c // 2, c % 2
        xT = np.ascontiguousarray(x[b_].T.astype(np.float64)).astype(BF16_NP)
        in_maps.append({
            "xT": xT,
            "xqT": np.ascontiguousarray(xT[:, half * LQ:(half + 1) * LQ]),
            "wqT": wqT, "wkT": wkT, "wvT": wvT, "woP": woP,
            "aq": np.ascontiguousarray(Aq[half * LQ:(half + 1) * LQ]),
            "bq": np.ascontiguousarray(Bq[half * LQ:(half + 1) * LQ]),
            "ak": Ak, "bk": Bk,
        })

    res = run_bass_kernel_spmd(nc, in_maps, core_ids=list(range(8)))
    out = np.empty((B, L, C), dtype=np.float32)
    for c in range(8):
        b_, half = c // 2, c % 2
        out[b_, half * LQ:(half + 1) * LQ] = res.results[c]["out"]
    return out
